# revision 11
# baseline (speedup 1.0000x reference)
"""DepthwiseXCorr (SiamRPN head) on 8 trn2 cores — PE-centric bf16 rewrite.

Data-parallel over batch: B=128 -> 16 samples/core. Per sample:
  branch(x) = BN2(pw1x1(ReLU6(BN1(dw3x3(x)))))  for kernel (7x7) and search (31x31)
  out = per-channel xcorr(search_feat 29x29, kernel_feat 5x5) -> 25x25

Engine mapping (per core):
  - dw conv (both branches): PE matmuls with HOST-PRECOMPUTED diagonal weight
    matrices (bf16), accumulating 9 taps in PSUM (fp32). Kernel branch is
    batched across all 16 samples (free dim 400).
  - BN bias + ReLU: fused into ScalarE PSUM eviction (Relu, bias=b1).
    min(.,6) on VectorE (tensor_scalar with immediate hits the 2x mode).
  - pw conv: PE bf16 matmuls (BN2 folded into weights), ScalarE eviction
    with bias into padded [29,30] bf16 feature tiles.
  - xcorr: 32 tiles (sample x channel-block) routed across engines:
      'P': PE diag-matmuls; diagonals built from an identity via
           per-partition scale (split VectorE/ScalarE); fp32 PSUM accum.
      'V': VectorE scalar_tensor_tensor taps in 4 short bf16 chains
           (7/6/6/6) combined in fp32 (keeps accumulation error low);
           odd-column windows read a 1-shifted copy to stay 4B-aligned.
      'W': ScalarE per-tap products (activation with per-partition
           scale) + VectorE tree-sum of the contiguous product tiles.
  - outputs DMA'd from SBUF fp32 tiles.

Host-side layouts are channel-major so every DMA is contiguous per
partition: kern [256,16,7,7], srch [256,16,31,33] (col-padded to an odd
row stride: 66B avoids the SBUF port conflicts a 64B stride causes for
PE moving-operand streaming), out [256,16,25,25].
"""

import numpy as np
import ml_dtypes

import concourse.bass as bass
import concourse.mybir as mybir
from concourse.tile import TileContext
from concourse.bass_utils import run_bass_kernel_spmd

F32 = mybir.dt.float32
BF16 = mybir.dt.bfloat16
AF = mybir.ActivationFunctionType
OP = mybir.AluOpType
BF_NP = ml_dtypes.bfloat16

B, C, KH, SH = 128, 256, 7, 31
N_CORES = 8
BPC = B // N_CORES          # 16 samples per core
G = 2                       # channel blocks of 128
EPS = 1e-5
HO_K, HO_S, HO_X = 5, 29, 25
SW = 33                     # padded search row width (odd stride avoids SBUF port conflicts)
NKF = BPC * HO_K * HO_K     # 400: kernel branch batched free dim

# bf16 param column offsets: [dw diag k (2g x 9t x 128) | dw diag s |
#                             pw k lhsT (4 x 128) | pw s lhsT | identity]
O_DWK, O_DWS = 0, 2304
O_PWK, O_PWS = 4608, 5120
O_ID = 5632
NBF = 5760
# fp32 param cols: b1k g0,g1 | b2k g0,g1 | b1s g0,g1 | b2s g0,g1
NP32 = 8

# dw psum row-chunks (output rows), pw chunks (flat cols), xcorr row-chunks
DW_CH = ((0, 16), (16, 29))         # 464 / 377 elems
PW_CH = ((0, 493), (493, 841))      # 17x29 / 12x29
X_CH = ((0, 20), (20, 25))          # 500 / 125 elems
V_CHAINS = ((0, 7), (7, 13), (13, 19), (19, 25))


def _routes():
    # Three xcorr routes balanced across engines (measured costs):
    #   'P': PE diag-matmuls (~11us PE) + ScalarE diag builds
    #   'V': VectorE scalar_tensor_tensor taps (~21us V)
    #   'W': ScalarE per-tap products (activation, per-partition scale) +
    #        VectorE tree-sum of contiguous product tiles
    # One PE tile in every sample so the Tensor engine never drains (HAM
    # re-throttles after idle windows); second tile alternates V/W, with two
    # extra P tiles to balance measured engine loads (PE 254 / V 352 / S 324).
    r = []
    for s in range(16):
        r.append('P')
        r.append('V' if s % 2 == 1 else 'W')
    return r


ROUTES = _routes()

_cache: dict = {}
LAST_RESULTS = None


def _fold_branch(dw_w, bn1, pw_w, pw_b, bn2):
    """Fold eval-mode BN into conv weights/biases (host, fp32)."""
    g1, b1, m1, v1 = bn1[0], bn1[1], bn1[2], bn1[3]
    inv1 = g1 / np.sqrt(v1 + EPS)
    shift1 = b1 - m1 * inv1
    dw = (dw_w[:, 0] * inv1[:, None, None]).reshape(C, 9).astype(np.float32)

    g2, b2, m2, v2 = bn2[0], bn2[1], bn2[2], bn2[3]
    inv2 = g2 / np.sqrt(v2 + EPS)
    shift2 = b2 - m2 * inv2
    W = (pw_w[:, :, 0, 0] * inv2[:, None]).astype(np.float32)   # (co, ci)
    bias2 = (pw_b * inv2 + shift2).astype(np.float32)

    lhsT = np.zeros((G, G, 128, 128), np.float32)
    for gi in range(G):
        for go in range(G):
            lhsT[gi, go] = W[go * 128:(go + 1) * 128, gi * 128:(gi + 1) * 128].T
    return dw, shift1.astype(np.float32), lhsT, bias2


def _split_waits(nc, keep=1):
    """Container walrus accepts only one sync-wait per instruction; move
    extras onto standalone EventSemaphore instructions just before the
    owner in its engine stream."""
    import bass_rust

    n = 0
    for bb in nc.m.functions[0].blocks:
        out = []
        for ins in bb.instructions:
            si = ins.sync_info
            if si is not None and len(si.on_wait) > keep:
                waits = list(si.on_wait)
                for w in waits[:-keep]:
                    n += 1
                    ev = mybir.InstEventSemaphore(
                        name=f"antsplitw_{n}", ins=[], outs=[])
                    ev.engine = ins.engine
                    ev.sync_info = bass_rust.SyncInfo(on_wait=[w], on_update=[])
                    out.append(ev)
                ins.sync_info = bass_rust.SyncInfo(
                    on_wait=waits[-keep:], on_update=list(si.on_update))
            out.append(ins)
        bb.instructions = out
    return n


def _build_nc():
    nc = bass.Bass()

    kern_h = nc.declare_dram_parameter("kern_in", [C, BPC, KH, KH], BF16, isOutput=False)
    srch_h = nc.declare_dram_parameter("srch_in", [C, BPC, SH, SW], BF16, isOutput=False)
    pbf_h = nc.declare_dram_parameter("prmbf", [128, NBF], BF16, isOutput=False)
    p32_h = nc.declare_dram_parameter("prm32", [128, NP32], F32, isOutput=False)
    out_h = nc.declare_dram_parameter("out", [C, BPC, HO_X, HO_X], F32, isOutput=True)

    with TileContext(nc) as tc:
        with (
            tc.tile_pool(name="const", bufs=1) as cpool,
            tc.tile_pool(name="kio", bufs=1) as kpool,
        ):
            pbf = cpool.tile([128, NBF], BF16)
            nc.sync.dma_start(out=pbf[:], in_=pbf_h[:])
            p32 = cpool.tile([128, NP32], F32)
            nc.sync.dma_start(out=p32[:], in_=p32_h[:])

            def bias(i):
                return p32[:, i:i + 1]

            def dwk_w(g, t):
                o = O_DWK + (g * 9 + t) * 128
                return pbf[:, o:o + 128]

            def dws_w(g, t):
                o = O_DWS + (g * 9 + t) * 128
                return pbf[:, o:o + 128]

            def pwk_w(gi, go):
                o = O_PWK + (gi * G + go) * 128
                return pbf[:, o:o + 128]

            def pws_w(gi, go):
                o = O_PWS + (gi * G + go) * 128
                return pbf[:, o:o + 128]

            ID = pbf[:, O_ID:O_ID + 128]

            # ---- kernel branch, all 16 samples batched (free dim 400) ----
            K2 = []
            with tc.tile_pool(name="psk", bufs=2, space="PSUM") as pskp:
                hks = []
                for g in range(G):
                    xk = kpool.tile([128, BPC, KH, KH], BF16, name=f"xk{g}")
                    nc.sync.dma_start(out=xk[:], in_=kern_h[128 * g:128 * (g + 1)])
                    psd = pskp.tile([128, NKF], F32, name="pskd")
                    for t in range(9):
                        u, v = t // 3, t % 3
                        nc.tensor.matmul(
                            psd[:].rearrange("p (s a b) -> p s a b", s=BPC, a=5),
                            dwk_w(g, t), xk[:, :, u:u + 5, v:v + 5],
                            start=(t == 0), stop=(t == 8))
                    hk0 = kpool.tile([128, NKF], BF16, name=f"hk0{g}")
                    nc.scalar.activation(hk0[:], psd[:], AF.Relu,
                                         bias=bias(0 + g), scale=1.0)
                    hk = kpool.tile([128, NKF], BF16, name=f"hk{g}")
                    nc.vector.tensor_scalar(hk[:], hk0[:], 6.0, None, OP.min)
                    hks.append(hk)
                for go in range(G):
                    psp = pskp.tile([128, NKF], F32, name="pskp")
                    for gi in range(G):
                        nc.tensor.matmul(psp[:], pwk_w(gi, go), hks[gi][:],
                                         start=(gi == 0), stop=(gi == 1))
                    k2 = kpool.tile([128, NKF], F32, name=f"k2{go}")
                    nc.scalar.activation(k2[:], psp[:], AF.Identity,
                                         bias=bias(2 + go), scale=1.0)
                    K2.append(k2)

            # ---- search branch + xcorr, per sample ----
            with (
                tc.tile_pool(name="sio", bufs=3) as spool,
                tc.tile_pool(name="hp", bufs=2) as hpool,
                tc.tile_pool(name="s2p", bufs=2) as s2pool,
                tc.tile_pool(name="dg", bufs=2) as dpool,
                tc.tile_pool(name="vx", bufs=2) as vxp,
                tc.tile_pool(name="ox", bufs=2) as oxp,
                tc.tile_pool(name="psd", bufs=1, space="PSUM") as psdp,
                tc.tile_pool(name="psp", bufs=1, space="PSUM") as pspp,
                tc.tile_pool(name="psx", bufs=2, space="PSUM") as psxp,
            ):
                pending = []

                def flush_pending():
                    for (ps_, gs_, pcs_) in pending:
                        oxf = oxp.tile([128, HO_X, HO_X], F32, name="oxf")
                        for ci_, (r0_, r1_) in enumerate(X_CH):
                            nc.scalar.activation(
                                oxf[:, r0_:r1_, :],
                                pcs_[ci_][:].rearrange(
                                    "p (a b) -> p a b", a=r1_ - r0_),
                                AF.Identity, bias=0.0, scale=1.0)
                        nc.sync.dma_start(
                            out=out_h[128 * gs_:128 * (gs_ + 1), ps_],
                            in_=oxf[:])
                    pending.clear()

                for s in range(BPC):
                    # dw conv + relu6 per block
                    h2s = []
                    for g in range(G):
                        xs = spool.tile([128, SH, SW], BF16, name=f"xs{g}")
                        nc.sync.dma_start(
                            out=xs[:], in_=srch_h[128 * g:128 * (g + 1), s])
                        pcs = []
                        for (r0, r1) in DW_CH:
                            ps = psdp.tile([128, (r1 - r0) * HO_S], F32,
                                           name=f"dw{r0}")
                            pcs.append(ps)
                        for t in range(9):
                            u, v = t // 3, t % 3
                            for ci, (r0, r1) in enumerate(DW_CH):
                                nc.tensor.matmul(
                                    pcs[ci][:].rearrange(
                                        "p (a b) -> p a b", a=r1 - r0),
                                    dws_w(g, t),
                                    xs[:, u + r0:u + r1, v:v + HO_S],
                                    start=(t == 0), stop=(t == 8))
                        h = hpool.tile([128, HO_S * HO_S], BF16, name=f"h{g}")
                        o = 0
                        for ci, (r0, r1) in enumerate(DW_CH):
                            n = (r1 - r0) * HO_S
                            nc.scalar.activation(h[:, o:o + n], pcs[ci][:],
                                                 AF.Relu, bias=bias(4 + g),
                                                 scale=1.0)
                            o += n
                        h2 = hpool.tile([128, HO_S * HO_S], BF16, name=f"h2{g}")
                        nc.vector.tensor_scalar(h2[:], h[:], 6.0, None, OP.min)
                        h2s.append(h2)

                    # pw conv -> padded S2 tiles
                    S2s = []
                    for go in range(G):
                        s2 = s2pool.tile([128, HO_S, 30], BF16, name=f"s2{go}")
                        for (c0, c1) in PW_CH:
                            ps = pspp.tile([128, c1 - c0], F32, name=f"pw{c0}")
                            for gi in range(G):
                                nc.tensor.matmul(ps[:], pws_w(gi, go),
                                                 h2s[gi][:, c0:c1],
                                                 start=(gi == 0), stop=(gi == 1))
                            r0, r1 = c0 // HO_S, c1 // HO_S
                            nc.scalar.activation(
                                s2[:, r0:r1, 0:HO_S],
                                ps[:].rearrange("p (a b) -> p a b", a=r1 - r0),
                                AF.Identity, bias=bias(6 + go), scale=1.0)
                        S2s.append(s2)

                    # previous sample's PE-route evictions first: ScalarE
                    # reaches them early, freeing PSUM before PE needs it
                    flush_pending()

                    # xcorr per block
                    for g in range(G):
                        idx = s * 2 + g
                        route = ROUTES[idx]
                        s2 = S2s[g]

                        def k2c(t):
                            return K2[g][:, s * 25 + t:s * 25 + t + 1]

                        if route == 'P':
                            diags = []
                            for t in range(25):
                                d = dpool.tile([128, 128], BF16, name=f"d{t}")
                                if idx % 2 == 0:
                                    nc.vector.tensor_scalar(
                                        d[:], ID, k2c(t), None, OP.mult)
                                else:
                                    nc.scalar.activation(
                                        d[:], ID, AF.Identity, bias=0.0,
                                        scale=k2c(t))
                                diags.append(d)
                            pcs = []
                            for (r0, r1) in X_CH:
                                pcs.append(psxp.tile(
                                    [128, (r1 - r0) * HO_X], F32,
                                    name=f"x{r0}"))
                            for t in range(25):
                                u, v = t // 5, t % 5
                                for ci, (r0, r1) in enumerate(X_CH):
                                    nc.tensor.matmul(
                                        pcs[ci][:].rearrange(
                                            "p (a b) -> p a b", a=r1 - r0),
                                        diags[t][:],
                                        s2[:, u + r0:u + r1, v:v + HO_X],
                                        start=(t == 0), stop=(t == 24))
                            pending.append((s, g, pcs))
                        elif route == 'V':
                            sh = s2pool.tile([128, HO_S, 30], BF16,
                                             name=f"sh{g}")
                            nc.vector.tensor_copy(sh[:, :, 0:28],
                                                  s2[:, :, 1:29])
                            accs = []
                            for ci, (t0, t1) in enumerate(V_CHAINS):
                                a = vxp.tile([128, HO_X, HO_X], BF16,
                                             name=f"va{ci}")
                                for t in range(t0, t1):
                                    u, v = t // 5, t % 5
                                    if v % 2 == 0:
                                        win = s2[:, u:u + 25, v:v + 25]
                                    else:
                                        win = sh[:, u:u + 25, v - 1:v + 24]
                                    if t == t0:
                                        nc.vector.tensor_scalar(
                                            a[:], win, k2c(t), None, OP.mult)
                                    else:
                                        nc.vector.scalar_tensor_tensor(
                                            a[:], win, k2c(t), a[:],
                                            OP.mult, OP.add)
                                accs.append(a)
                            c01 = vxp.tile([128, HO_X, HO_X], BF16, name="c01")
                            nc.vector.tensor_tensor(
                                c01[:], accs[0][:], accs[1][:], OP.add)
                            c23 = vxp.tile([128, HO_X, HO_X], BF16, name="c23")
                            nc.vector.tensor_tensor(
                                c23[:], accs[2][:], accs[3][:], OP.add)
                            ovf = oxp.tile([128, HO_X, HO_X], F32, name="ovf")
                            nc.vector.tensor_tensor(
                                ovf[:], c01[:], c23[:], OP.add)
                            nc.sync.dma_start(
                                out=out_h[128 * g:128 * (g + 1), s],
                                in_=ovf[:])
                        else:  # 'W': ScalarE products + VectorE tree-sum
                            prods = []
                            for t in range(25):
                                u, v = t // 5, t % 5
                                pr = vxp.tile([128, HO_X * HO_X], BF16,
                                              name=f"pr{t % 8}")
                                nc.scalar.activation(
                                    pr[:].rearrange("p (a b) -> p a b", a=25),
                                    s2[:, u:u + 25, v:v + 25],
                                    AF.Identity, bias=0.0, scale=k2c(t))
                                prods.append(pr)
                            caccs = []
                            for ci, (t0, t1) in enumerate(V_CHAINS):
                                ca = vxp.tile([128, HO_X * HO_X], BF16,
                                              name=f"wc{ci}")
                                nc.vector.tensor_tensor(
                                    ca[:], prods[t0][:], prods[t0 + 1][:],
                                    OP.add)
                                for t in range(t0 + 2, t1):
                                    nc.vector.tensor_tensor(
                                        ca[:], ca[:], prods[t][:], OP.add)
                                caccs.append(ca)
                            c01 = vxp.tile([128, HO_X * HO_X], BF16,
                                           name="wc01")
                            nc.vector.tensor_tensor(
                                c01[:], caccs[0][:], caccs[1][:], OP.add)
                            c23 = vxp.tile([128, HO_X * HO_X], BF16,
                                           name="wc23")
                            nc.vector.tensor_tensor(
                                c23[:], caccs[2][:], caccs[3][:], OP.add)
                            ovf = oxp.tile([128, HO_X, HO_X], F32, name="ovf")
                            nc.vector.tensor_tensor(
                                ovf[:].rearrange("p a b -> p (a b)"),
                                c01[:], c23[:], OP.add)
                            nc.sync.dma_start(
                                out=out_h[128 * g:128 * (g + 1), s],
                                in_=ovf[:])
                flush_pending()
    _split_waits(nc)
    return nc


def _pack_params(kdw, ks1, kpw, kb2, sdw, ss1, spw, sb2):
    pbf = np.zeros((128, NBF), np.float32)
    for g in range(G):
        for t in range(9):
            d = np.diag(kdw[g * 128:(g + 1) * 128, t])
            pbf[:, O_DWK + (g * 9 + t) * 128:O_DWK + (g * 9 + t + 1) * 128] = d
            d = np.diag(sdw[g * 128:(g + 1) * 128, t])
            pbf[:, O_DWS + (g * 9 + t) * 128:O_DWS + (g * 9 + t + 1) * 128] = d
    for gi in range(G):
        for go in range(G):
            o = O_PWK + (gi * G + go) * 128
            pbf[:, o:o + 128] = kpw[gi, go]
            o = O_PWS + (gi * G + go) * 128
            pbf[:, o:o + 128] = spw[gi, go]
    pbf[:, O_ID:O_ID + 128] = np.eye(128, dtype=np.float32)

    p32 = np.zeros((128, NP32), np.float32)
    for g in range(G):
        p32[:, 0 + g] = ks1[g * 128:(g + 1) * 128]
        p32[:, 2 + g] = kb2[g * 128:(g + 1) * 128]
        p32[:, 4 + g] = ss1[g * 128:(g + 1) * 128]
        p32[:, 6 + g] = sb2[g * 128:(g + 1) * 128]
    return pbf.astype(BF_NP), p32


def kernel(kernel, search, k_dw_w, k_bn1, k_pw_w, k_pw_b, k_bn2,
           s_dw_w, s_bn1, s_pw_w, s_pw_b, s_bn2):
    global LAST_RESULTS
    kdw, ks1, kpw, kb2 = _fold_branch(np.asarray(k_dw_w), np.asarray(k_bn1),
                                      np.asarray(k_pw_w), np.asarray(k_pw_b),
                                      np.asarray(k_bn2))
    sdw, ss1, spw, sb2 = _fold_branch(np.asarray(s_dw_w), np.asarray(s_bn1),
                                      np.asarray(s_pw_w), np.asarray(s_pw_b),
                                      np.asarray(s_bn2))
    pbf, p32 = _pack_params(kdw, ks1, kpw, kb2, sdw, ss1, spw, sb2)

    kern = np.asarray(kernel, np.float32)
    srch = np.asarray(search, np.float32)
    # channel-major per-core layouts, search col-padded 31->32
    kern_cm = np.ascontiguousarray(
        kern.reshape(N_CORES, BPC, C, KH, KH).transpose(0, 2, 1, 3, 4)
    ).astype(BF_NP)
    srch_p = np.zeros((N_CORES, C, BPC, SH, SW), np.float32)
    srch_p[..., :SH] = srch.reshape(N_CORES, BPC, C, SH, SH).transpose(
        0, 2, 1, 3, 4)
    srch_cm = srch_p.astype(BF_NP)

    if "nc" not in _cache:
        _cache["nc"] = _build_nc()
    nc = _cache["nc"]

    in_maps = []
    for i in range(N_CORES):
        in_maps.append({"kern_in": kern_cm[i], "srch_in": srch_cm[i],
                        "prmbf": pbf, "prm32": p32})

    res = run_bass_kernel_spmd(nc, in_maps, list(range(N_CORES)))
    LAST_RESULTS = res
    outs = []
    for i in range(N_CORES):
        o = res.results[i]["out"]          # [C, BPC, 25, 25]
        outs.append(np.ascontiguousarray(o.transpose(1, 0, 2, 3)))
    return np.concatenate(outs, axis=0)


# revision 12
# speedup vs baseline: 1.0242x; 1.0242x over previous
"""DepthwiseXCorr (SiamRPN head) on 8 trn2 cores — PE-centric bf16 rewrite.

Data-parallel over batch: B=128 -> 16 samples/core. Per sample:
  branch(x) = BN2(pw1x1(ReLU6(BN1(dw3x3(x)))))  for kernel (7x7) and search (31x31)
  out = per-channel xcorr(search_feat 29x29, kernel_feat 5x5) -> 25x25

Engine mapping (per core):
  - dw conv (both branches): PE matmuls with HOST-PRECOMPUTED diagonal weight
    matrices (bf16), accumulating 9 taps in PSUM (fp32). Kernel branch is
    batched across all 16 samples (free dim 400).
  - BN bias + ReLU: fused into ScalarE PSUM eviction (Relu, bias=b1).
    min(.,6) on VectorE (tensor_scalar with immediate hits the 2x mode).
  - pw conv: PE bf16 matmuls (BN2 folded into weights), ScalarE eviction
    with bias into padded [29,30] bf16 feature tiles.
  - xcorr: 32 tiles (sample x channel-block) routed across engines:
      'P': PE diag-matmuls; diagonals built from an identity via
           per-partition scale (split VectorE/ScalarE); fp32 PSUM accum.
      'V': VectorE scalar_tensor_tensor taps in 4 short bf16 chains
           (7/6/6/6) combined in fp32 (keeps accumulation error low);
           odd-column windows read a 1-shifted copy to stay 4B-aligned.
      'W': ScalarE per-tap products (activation with per-partition
           scale) + VectorE tree-sum of the contiguous product tiles.
  - outputs DMA'd from SBUF fp32 tiles.

Host-side layouts are channel-major so every DMA is contiguous per
partition: kern [256,16,7,7], srch [256,16,31,33] (col-padded to an odd
row stride: 66B avoids the SBUF port conflicts a 64B stride causes for
PE moving-operand streaming), out [256,16,25,25].
"""

import numpy as np
import ml_dtypes

import concourse.bass as bass
import concourse.mybir as mybir
from concourse.tile import TileContext
from concourse.bass_utils import run_bass_kernel_spmd

F32 = mybir.dt.float32
BF16 = mybir.dt.bfloat16
AF = mybir.ActivationFunctionType
OP = mybir.AluOpType
BF_NP = ml_dtypes.bfloat16

B, C, KH, SH = 128, 256, 7, 31
N_CORES = 8
BPC = B // N_CORES          # 16 samples per core
G = 2                       # channel blocks of 128
EPS = 1e-5
HO_K, HO_S, HO_X = 5, 29, 25
SW = 33                     # padded search row width (odd stride avoids SBUF port conflicts)
NKF = BPC * HO_K * HO_K     # 400: kernel branch batched free dim

# bf16 param column offsets: [dw diag k (2g x 9t x 128) | dw diag s |
#                             pw k lhsT (4 x 128) | pw s lhsT | identity]
O_DWK, O_DWS = 0, 2304
O_PWK, O_PWS = 4608, 5120
O_ID = 5632
NBF = 5760
# fp32 param cols: b1k g0,g1 | b2k g0,g1 | b1s g0,g1 | b2s g0,g1
NP32 = 8

# dw psum row-chunks (output rows), pw chunks (flat cols), xcorr row-chunks
DW_CH = ((0, 16), (16, 29))         # 464 / 377 elems
PW_CH = ((0, 493), (493, 841))      # 17x29 / 12x29
X_CH = ((0, 20), (20, 25))          # 500 / 125 elems
V_CHAINS = ((0, 7), (7, 13), (13, 19), (19, 25))


def _routes():
    # Three xcorr routes balanced across engines (measured costs):
    #   'P': PE diag-matmuls (~11us PE) + ScalarE diag builds
    #   'V': VectorE scalar_tensor_tensor taps (~21us V)
    #   'W': ScalarE per-tap products (activation, per-partition scale) +
    #        VectorE tree-sum of contiguous product tiles
    # One PE tile in every sample so the Tensor engine never drains (HAM
    # re-throttles after idle windows); second tile alternates V/W, with two
    # extra P tiles to balance measured engine loads (PE 254 / V 352 / S 324).
    r = []
    for s in range(16):
        r.append('P')
        if s in (0, 8):
            r.append('P')
        elif s % 2 == 1:
            r.append('V')
        else:
            r.append('W')
    return r


ROUTES = _routes()

_cache: dict = {}
LAST_RESULTS = None


def _fold_branch(dw_w, bn1, pw_w, pw_b, bn2):
    """Fold eval-mode BN into conv weights/biases (host, fp32)."""
    g1, b1, m1, v1 = bn1[0], bn1[1], bn1[2], bn1[3]
    inv1 = g1 / np.sqrt(v1 + EPS)
    shift1 = b1 - m1 * inv1
    dw = (dw_w[:, 0] * inv1[:, None, None]).reshape(C, 9).astype(np.float32)

    g2, b2, m2, v2 = bn2[0], bn2[1], bn2[2], bn2[3]
    inv2 = g2 / np.sqrt(v2 + EPS)
    shift2 = b2 - m2 * inv2
    W = (pw_w[:, :, 0, 0] * inv2[:, None]).astype(np.float32)   # (co, ci)
    bias2 = (pw_b * inv2 + shift2).astype(np.float32)

    lhsT = np.zeros((G, G, 128, 128), np.float32)
    for gi in range(G):
        for go in range(G):
            lhsT[gi, go] = W[go * 128:(go + 1) * 128, gi * 128:(gi + 1) * 128].T
    return dw, shift1.astype(np.float32), lhsT, bias2


def _split_waits(nc, keep=1):
    """Container walrus accepts only one sync-wait per instruction; move
    extras onto standalone EventSemaphore instructions just before the
    owner in its engine stream."""
    import bass_rust

    n = 0
    for bb in nc.m.functions[0].blocks:
        out = []
        for ins in bb.instructions:
            si = ins.sync_info
            if si is not None and len(si.on_wait) > keep:
                waits = list(si.on_wait)
                for w in waits[:-keep]:
                    n += 1
                    ev = mybir.InstEventSemaphore(
                        name=f"antsplitw_{n}", ins=[], outs=[])
                    ev.engine = ins.engine
                    ev.sync_info = bass_rust.SyncInfo(on_wait=[w], on_update=[])
                    out.append(ev)
                ins.sync_info = bass_rust.SyncInfo(
                    on_wait=waits[-keep:], on_update=list(si.on_update))
            out.append(ins)
        bb.instructions = out
    return n


def _build_nc():
    nc = bass.Bass()

    kern_h = nc.declare_dram_parameter("kern_in", [C, BPC, KH, KH], BF16, isOutput=False)
    srch_h = nc.declare_dram_parameter("srch_in", [C, BPC, SH, SW], BF16, isOutput=False)
    pbf_h = nc.declare_dram_parameter("prmbf", [128, NBF], BF16, isOutput=False)
    p32_h = nc.declare_dram_parameter("prm32", [128, NP32], F32, isOutput=False)
    out_h = nc.declare_dram_parameter("out", [C, BPC, HO_X, HO_X], F32, isOutput=True)

    with TileContext(nc) as tc:
        with (
            tc.tile_pool(name="const", bufs=1) as cpool,
            tc.tile_pool(name="kio", bufs=1) as kpool,
        ):
            pbf = cpool.tile([128, NBF], BF16)
            nc.sync.dma_start(out=pbf[:], in_=pbf_h[:])
            p32 = cpool.tile([128, NP32], F32)
            nc.sync.dma_start(out=p32[:], in_=p32_h[:])

            def bias(i):
                return p32[:, i:i + 1]

            def dwk_w(g, t):
                o = O_DWK + (g * 9 + t) * 128
                return pbf[:, o:o + 128]

            def dws_w(g, t):
                o = O_DWS + (g * 9 + t) * 128
                return pbf[:, o:o + 128]

            def pwk_w(gi, go):
                o = O_PWK + (gi * G + go) * 128
                return pbf[:, o:o + 128]

            def pws_w(gi, go):
                o = O_PWS + (gi * G + go) * 128
                return pbf[:, o:o + 128]

            ID = pbf[:, O_ID:O_ID + 128]

            # ---- kernel branch, all 16 samples batched (free dim 400) ----
            K2 = []
            with tc.tile_pool(name="psk", bufs=2, space="PSUM") as pskp:
                hks = []
                for g in range(G):
                    xk = kpool.tile([128, BPC, KH, KH], BF16, name=f"xk{g}")
                    nc.sync.dma_start(out=xk[:], in_=kern_h[128 * g:128 * (g + 1)])
                    psd = pskp.tile([128, NKF], F32, name="pskd")
                    for t in range(9):
                        u, v = t // 3, t % 3
                        nc.tensor.matmul(
                            psd[:].rearrange("p (s a b) -> p s a b", s=BPC, a=5),
                            dwk_w(g, t), xk[:, :, u:u + 5, v:v + 5],
                            start=(t == 0), stop=(t == 8))
                    hk0 = kpool.tile([128, NKF], BF16, name=f"hk0{g}")
                    nc.scalar.activation(hk0[:], psd[:], AF.Relu,
                                         bias=bias(0 + g), scale=1.0)
                    hk = kpool.tile([128, NKF], BF16, name=f"hk{g}")
                    nc.vector.tensor_scalar(hk[:], hk0[:], 6.0, None, OP.min)
                    hks.append(hk)
                for go in range(G):
                    psp = pskp.tile([128, NKF], F32, name="pskp")
                    for gi in range(G):
                        nc.tensor.matmul(psp[:], pwk_w(gi, go), hks[gi][:],
                                         start=(gi == 0), stop=(gi == 1))
                    k2 = kpool.tile([128, NKF], F32, name=f"k2{go}")
                    nc.scalar.activation(k2[:], psp[:], AF.Identity,
                                         bias=bias(2 + go), scale=1.0)
                    K2.append(k2)

            # ---- search branch + xcorr, per sample ----
            with (
                tc.tile_pool(name="sio", bufs=3) as spool,
                tc.tile_pool(name="hp", bufs=2) as hpool,
                tc.tile_pool(name="s2p", bufs=2) as s2pool,
                tc.tile_pool(name="dg", bufs=2) as dpool,
                tc.tile_pool(name="vx", bufs=2) as vxp,
                tc.tile_pool(name="ox", bufs=2) as oxp,
                tc.tile_pool(name="psd", bufs=1, space="PSUM") as psdp,
                tc.tile_pool(name="psp", bufs=1, space="PSUM") as pspp,
                tc.tile_pool(name="psx", bufs=2, space="PSUM") as psxp,
            ):
                for s in range(BPC):
                    # dw conv + relu6 per block
                    h2s = []
                    for g in range(G):
                        xs = spool.tile([128, SH, SW], BF16, name=f"xs{g}")
                        nc.sync.dma_start(
                            out=xs[:], in_=srch_h[128 * g:128 * (g + 1), s])
                        pcs = []
                        for (r0, r1) in DW_CH:
                            ps = psdp.tile([128, (r1 - r0) * HO_S], F32,
                                           name=f"dw{r0}")
                            pcs.append(ps)
                        for t in range(9):
                            u, v = t // 3, t % 3
                            for ci, (r0, r1) in enumerate(DW_CH):
                                nc.tensor.matmul(
                                    pcs[ci][:].rearrange(
                                        "p (a b) -> p a b", a=r1 - r0),
                                    dws_w(g, t),
                                    xs[:, u + r0:u + r1, v:v + HO_S],
                                    start=(t == 0), stop=(t == 8))
                        h = hpool.tile([128, HO_S * HO_S], BF16, name=f"h{g}")
                        o = 0
                        for ci, (r0, r1) in enumerate(DW_CH):
                            n = (r1 - r0) * HO_S
                            nc.scalar.activation(h[:, o:o + n], pcs[ci][:],
                                                 AF.Relu, bias=bias(4 + g),
                                                 scale=1.0)
                            o += n
                        h2 = hpool.tile([128, HO_S * HO_S], BF16, name=f"h2{g}")
                        nc.vector.tensor_scalar(h2[:], h[:], 6.0, None, OP.min)
                        h2s.append(h2)

                    # pw conv -> padded S2 tiles
                    S2s = []
                    for go in range(G):
                        s2 = s2pool.tile([128, HO_S, 30], BF16, name=f"s2{go}")
                        for (c0, c1) in PW_CH:
                            ps = pspp.tile([128, c1 - c0], F32, name=f"pw{c0}")
                            for gi in range(G):
                                nc.tensor.matmul(ps[:], pws_w(gi, go),
                                                 h2s[gi][:, c0:c1],
                                                 start=(gi == 0), stop=(gi == 1))
                            r0, r1 = c0 // HO_S, c1 // HO_S
                            nc.scalar.activation(
                                s2[:, r0:r1, 0:HO_S],
                                ps[:].rearrange("p (a b) -> p a b", a=r1 - r0),
                                AF.Identity, bias=bias(6 + go), scale=1.0)
                        S2s.append(s2)

                    # xcorr per block
                    for g in range(G):
                        idx = s * 2 + g
                        route = ROUTES[idx]
                        s2 = S2s[g]

                        def k2c(t):
                            return K2[g][:, s * 25 + t:s * 25 + t + 1]

                        if route == 'P':
                            diags = []
                            for t in range(25):
                                d = dpool.tile([128, 128], BF16, name=f"d{t}")
                                if idx % 3 != 2:
                                    nc.vector.tensor_scalar(
                                        d[:], ID, k2c(t), None, OP.mult)
                                else:
                                    nc.scalar.activation(
                                        d[:], ID, AF.Identity, bias=0.0,
                                        scale=k2c(t))
                                diags.append(d)
                            pcs = []
                            for (r0, r1) in X_CH:
                                pcs.append(psxp.tile(
                                    [128, (r1 - r0) * HO_X], F32,
                                    name=f"x{r0}"))
                            for t in range(25):
                                u, v = t // 5, t % 5
                                for ci, (r0, r1) in enumerate(X_CH):
                                    nc.tensor.matmul(
                                        pcs[ci][:].rearrange(
                                            "p (a b) -> p a b", a=r1 - r0),
                                        diags[t][:],
                                        s2[:, u + r0:u + r1, v:v + HO_X],
                                        start=(t == 0), stop=(t == 24))
                            oxf = oxp.tile([128, HO_X, HO_X], F32, name="oxf")
                            for ci, (r0, r1) in enumerate(X_CH):
                                nc.vector.tensor_copy(
                                    oxf[:, r0:r1, :],
                                    pcs[ci][:].rearrange(
                                        "p (a b) -> p a b", a=r1 - r0))
                            nc.sync.dma_start(
                                out=out_h[128 * g:128 * (g + 1), s],
                                in_=oxf[:])
                        elif route == 'V':
                            sh = s2pool.tile([128, HO_S, 30], BF16,
                                             name=f"sh{g}")
                            nc.vector.tensor_copy(sh[:, :, 0:28],
                                                  s2[:, :, 1:29])
                            accs = []
                            for ci, (t0, t1) in enumerate(V_CHAINS):
                                a = vxp.tile([128, HO_X, HO_X], BF16,
                                             name=f"va{ci}")
                                for t in range(t0, t1):
                                    u, v = t // 5, t % 5
                                    if v % 2 == 0:
                                        win = s2[:, u:u + 25, v:v + 25]
                                    else:
                                        win = sh[:, u:u + 25, v - 1:v + 24]
                                    if t == t0:
                                        nc.vector.tensor_scalar(
                                            a[:], win, k2c(t), None, OP.mult)
                                    else:
                                        nc.vector.scalar_tensor_tensor(
                                            a[:], win, k2c(t), a[:],
                                            OP.mult, OP.add)
                                accs.append(a)
                            c01 = vxp.tile([128, HO_X, HO_X], BF16, name="c01")
                            nc.vector.tensor_tensor(
                                c01[:], accs[0][:], accs[1][:], OP.add)
                            c23 = vxp.tile([128, HO_X, HO_X], BF16, name="c23")
                            nc.vector.tensor_tensor(
                                c23[:], accs[2][:], accs[3][:], OP.add)
                            ovf = oxp.tile([128, HO_X, HO_X], F32, name="ovf")
                            nc.vector.tensor_tensor(
                                ovf[:], c01[:], c23[:], OP.add)
                            nc.sync.dma_start(
                                out=out_h[128 * g:128 * (g + 1), s],
                                in_=ovf[:])
                        else:  # 'W': ScalarE products + VectorE tree-sum
                            prods = []
                            for t in range(25):
                                u, v = t // 5, t % 5
                                pr = vxp.tile([128, HO_X * HO_X], BF16,
                                              name=f"pr{t % 8}")
                                nc.scalar.activation(
                                    pr[:].rearrange("p (a b) -> p a b", a=25),
                                    s2[:, u:u + 25, v:v + 25],
                                    AF.Identity, bias=0.0, scale=k2c(t))
                                prods.append(pr)
                            caccs = []
                            for ci, (t0, t1) in enumerate(V_CHAINS):
                                ca = vxp.tile([128, HO_X * HO_X], BF16,
                                              name=f"wc{ci}")
                                nc.vector.tensor_tensor(
                                    ca[:], prods[t0][:], prods[t0 + 1][:],
                                    OP.add)
                                for t in range(t0 + 2, t1):
                                    nc.vector.tensor_tensor(
                                        ca[:], ca[:], prods[t][:], OP.add)
                                caccs.append(ca)
                            c01 = vxp.tile([128, HO_X * HO_X], BF16,
                                           name="wc01")
                            nc.vector.tensor_tensor(
                                c01[:], caccs[0][:], caccs[1][:], OP.add)
                            c23 = vxp.tile([128, HO_X * HO_X], BF16,
                                           name="wc23")
                            nc.vector.tensor_tensor(
                                c23[:], caccs[2][:], caccs[3][:], OP.add)
                            ovf = oxp.tile([128, HO_X, HO_X], F32, name="ovf")
                            nc.vector.tensor_tensor(
                                ovf[:].rearrange("p a b -> p (a b)"),
                                c01[:], c23[:], OP.add)
                            nc.sync.dma_start(
                                out=out_h[128 * g:128 * (g + 1), s],
                                in_=ovf[:])
    _split_waits(nc)
    return nc


def _pack_params(kdw, ks1, kpw, kb2, sdw, ss1, spw, sb2):
    pbf = np.zeros((128, NBF), np.float32)
    for g in range(G):
        for t in range(9):
            d = np.diag(kdw[g * 128:(g + 1) * 128, t])
            pbf[:, O_DWK + (g * 9 + t) * 128:O_DWK + (g * 9 + t + 1) * 128] = d
            d = np.diag(sdw[g * 128:(g + 1) * 128, t])
            pbf[:, O_DWS + (g * 9 + t) * 128:O_DWS + (g * 9 + t + 1) * 128] = d
    for gi in range(G):
        for go in range(G):
            o = O_PWK + (gi * G + go) * 128
            pbf[:, o:o + 128] = kpw[gi, go]
            o = O_PWS + (gi * G + go) * 128
            pbf[:, o:o + 128] = spw[gi, go]
    pbf[:, O_ID:O_ID + 128] = np.eye(128, dtype=np.float32)

    p32 = np.zeros((128, NP32), np.float32)
    for g in range(G):
        p32[:, 0 + g] = ks1[g * 128:(g + 1) * 128]
        p32[:, 2 + g] = kb2[g * 128:(g + 1) * 128]
        p32[:, 4 + g] = ss1[g * 128:(g + 1) * 128]
        p32[:, 6 + g] = sb2[g * 128:(g + 1) * 128]
    return pbf.astype(BF_NP), p32


def kernel(kernel, search, k_dw_w, k_bn1, k_pw_w, k_pw_b, k_bn2,
           s_dw_w, s_bn1, s_pw_w, s_pw_b, s_bn2):
    global LAST_RESULTS
    kdw, ks1, kpw, kb2 = _fold_branch(np.asarray(k_dw_w), np.asarray(k_bn1),
                                      np.asarray(k_pw_w), np.asarray(k_pw_b),
                                      np.asarray(k_bn2))
    sdw, ss1, spw, sb2 = _fold_branch(np.asarray(s_dw_w), np.asarray(s_bn1),
                                      np.asarray(s_pw_w), np.asarray(s_pw_b),
                                      np.asarray(s_bn2))
    pbf, p32 = _pack_params(kdw, ks1, kpw, kb2, sdw, ss1, spw, sb2)

    kern = np.asarray(kernel, np.float32)
    srch = np.asarray(search, np.float32)
    # channel-major per-core layouts, search col-padded 31->32
    kern_cm = np.ascontiguousarray(
        kern.reshape(N_CORES, BPC, C, KH, KH).transpose(0, 2, 1, 3, 4)
    ).astype(BF_NP)
    srch_p = np.zeros((N_CORES, C, BPC, SH, SW), np.float32)
    srch_p[..., :SH] = srch.reshape(N_CORES, BPC, C, SH, SH).transpose(
        0, 2, 1, 3, 4)
    srch_cm = srch_p.astype(BF_NP)

    if "nc" not in _cache:
        _cache["nc"] = _build_nc()
    nc = _cache["nc"]

    in_maps = []
    for i in range(N_CORES):
        in_maps.append({"kern_in": kern_cm[i], "srch_in": srch_cm[i],
                        "prmbf": pbf, "prm32": p32})

    res = run_bass_kernel_spmd(nc, in_maps, list(range(N_CORES)))
    LAST_RESULTS = res
    outs = []
    for i in range(N_CORES):
        o = res.results[i]["out"]          # [C, BPC, 25, 25]
        outs.append(np.ascontiguousarray(o.transpose(1, 0, 2, 3)))
    return np.concatenate(outs, axis=0)


# revision 13
# speedup vs baseline: 1.0468x; 1.0221x over previous
"""DepthwiseXCorr (SiamRPN head) on 8 trn2 cores — PE-centric bf16 rewrite.

Data-parallel over batch: B=128 -> 16 samples/core. Per sample:
  branch(x) = BN2(pw1x1(ReLU6(BN1(dw3x3(x)))))  for kernel (7x7) and search (31x31)
  out = per-channel xcorr(search_feat 29x29, kernel_feat 5x5) -> 25x25

Engine mapping (per core):
  - dw conv (both branches): PE matmuls with HOST-PRECOMPUTED diagonal weight
    matrices (bf16), accumulating 9 taps in PSUM (fp32). Kernel branch is
    batched across all 16 samples (free dim 400).
  - BN bias + ReLU: fused into ScalarE PSUM eviction (Relu, bias=b1).
    min(.,6) on VectorE (tensor_scalar with immediate hits the 2x mode).
  - pw conv: PE bf16 matmuls (BN2 folded into weights), ScalarE eviction
    with bias into padded [29,30] bf16 feature tiles.
  - xcorr: 32 tiles (sample x channel-block) routed across engines:
      'P': PE diag-matmuls; diagonals built from an identity via
           per-partition scale (split VectorE/ScalarE); fp32 PSUM accum.
      'V': VectorE scalar_tensor_tensor taps in 4 short bf16 chains
           (7/6/6/6) combined in fp32 (keeps accumulation error low);
           odd-column windows read a 1-shifted copy to stay 4B-aligned.
      'W': ScalarE per-tap products (activation with per-partition
           scale) + VectorE tree-sum of the contiguous product tiles.
  - outputs DMA'd from SBUF fp32 tiles.

Host-side layouts are channel-major so every DMA is contiguous per
partition: kern [256,16,7,7], srch [256,16,31,33] (col-padded to an odd
row stride: 66B avoids the SBUF port conflicts a 64B stride causes for
PE moving-operand streaming), out [256,16,25,25].
"""

import numpy as np
import ml_dtypes

import concourse.bass as bass
import concourse.mybir as mybir
from concourse.tile import TileContext
from concourse.bass_utils import run_bass_kernel_spmd

F32 = mybir.dt.float32
BF16 = mybir.dt.bfloat16
AF = mybir.ActivationFunctionType
OP = mybir.AluOpType
BF_NP = ml_dtypes.bfloat16

B, C, KH, SH = 128, 256, 7, 31
N_CORES = 8
BPC = B // N_CORES          # 16 samples per core
G = 2                       # channel blocks of 128
EPS = 1e-5
HO_K, HO_S, HO_X = 5, 29, 25
SW = 33                     # padded search row width (odd stride avoids SBUF port conflicts)
NKF = BPC * HO_K * HO_K     # 400: kernel branch batched free dim

# bf16 param column offsets: [dw diag k (2g x 9t x 128) | dw diag s |
#                             pw k lhsT (4 x 128) | pw s lhsT | identity]
O_DWK, O_DWS = 0, 2304
O_PWK, O_PWS = 4608, 5120
O_ID = 5632
NBF = 5760
# fp32 param cols: b1k g0,g1 | b2k g0,g1 | b1s g0,g1 | b2s g0,g1
NP32 = 8

# dw psum row-chunks (output rows), pw chunks (flat cols), xcorr row-chunks
DW_CH = ((0, 16), (16, 29))         # 464 / 377 elems
PW_CH = ((0, 493), (493, 841))      # 17x29 / 12x29
X_CH = ((0, 20), (20, 25))          # 500 / 125 elems
V_CHAINS = ((0, 7), (7, 13), (13, 19), (19, 25))


def _routes():
    # Three xcorr routes balanced across engines (measured costs):
    #   'P': PE diag-matmuls (~11us PE) + ScalarE diag builds
    #   'V': VectorE scalar_tensor_tensor taps (~21us V)
    #   'W': ScalarE per-tap products (activation, per-partition scale) +
    #        VectorE tree-sum of contiguous product tiles
    # One PE tile in every sample so the Tensor engine never drains (HAM
    # re-throttles after idle windows); second tile alternates V/W, with two
    # extra P tiles to balance measured engine loads (PE 254 / V 352 / S 324).
    r = []
    for s in range(16):
        r.append('P')
        if s in (0, 8):
            r.append('P')
        elif s % 2 == 1:
            r.append('V')
        else:
            r.append('W')
    return r


ROUTES = _routes()

_cache: dict = {}
LAST_RESULTS = None


def _fold_branch(dw_w, bn1, pw_w, pw_b, bn2):
    """Fold eval-mode BN into conv weights/biases (host, fp32)."""
    g1, b1, m1, v1 = bn1[0], bn1[1], bn1[2], bn1[3]
    inv1 = g1 / np.sqrt(v1 + EPS)
    shift1 = b1 - m1 * inv1
    dw = (dw_w[:, 0] * inv1[:, None, None]).reshape(C, 9).astype(np.float32)

    g2, b2, m2, v2 = bn2[0], bn2[1], bn2[2], bn2[3]
    inv2 = g2 / np.sqrt(v2 + EPS)
    shift2 = b2 - m2 * inv2
    W = (pw_w[:, :, 0, 0] * inv2[:, None]).astype(np.float32)   # (co, ci)
    bias2 = (pw_b * inv2 + shift2).astype(np.float32)

    lhsT = np.zeros((G, G, 128, 128), np.float32)
    for gi in range(G):
        for go in range(G):
            lhsT[gi, go] = W[go * 128:(go + 1) * 128, gi * 128:(gi + 1) * 128].T
    return dw, shift1.astype(np.float32), lhsT, bias2


def _split_waits(nc, keep=1):
    """Container walrus accepts only one sync-wait per instruction; move
    extras onto standalone EventSemaphore instructions just before the
    owner in its engine stream."""
    import bass_rust

    n = 0
    for bb in nc.m.functions[0].blocks:
        out = []
        for ins in bb.instructions:
            si = ins.sync_info
            if si is not None and len(si.on_wait) > keep:
                waits = list(si.on_wait)
                for w in waits[:-keep]:
                    n += 1
                    ev = mybir.InstEventSemaphore(
                        name=f"antsplitw_{n}", ins=[], outs=[])
                    ev.engine = ins.engine
                    ev.sync_info = bass_rust.SyncInfo(on_wait=[w], on_update=[])
                    out.append(ev)
                ins.sync_info = bass_rust.SyncInfo(
                    on_wait=waits[-keep:], on_update=list(si.on_update))
            out.append(ins)
        bb.instructions = out
    return n


def _build_nc():
    nc = bass.Bass()

    kern_h = nc.declare_dram_parameter("kern_in", [C, BPC, KH, KH], BF16, isOutput=False)
    srch_h = nc.declare_dram_parameter("srch_in", [C, BPC, SH, SW], BF16, isOutput=False)
    pbf_h = nc.declare_dram_parameter("prmbf", [128, NBF], BF16, isOutput=False)
    p32_h = nc.declare_dram_parameter("prm32", [128, NP32], F32, isOutput=False)
    out_h = nc.declare_dram_parameter("out", [C, BPC, HO_X, HO_X], F32, isOutput=True)

    with TileContext(nc) as tc:
        with (
            tc.tile_pool(name="const", bufs=1) as cpool,
            tc.tile_pool(name="kio", bufs=1) as kpool,
        ):
            pbf = cpool.tile([128, NBF], BF16)
            nc.sync.dma_start(out=pbf[:], in_=pbf_h[:])
            p32 = cpool.tile([128, NP32], F32)
            nc.sync.dma_start(out=p32[:], in_=p32_h[:])

            def bias(i):
                return p32[:, i:i + 1]

            def dwk_w(g, t):
                o = O_DWK + (g * 9 + t) * 128
                return pbf[:, o:o + 128]

            def dws_w(g, t):
                o = O_DWS + (g * 9 + t) * 128
                return pbf[:, o:o + 128]

            def pwk_w(gi, go):
                o = O_PWK + (gi * G + go) * 128
                return pbf[:, o:o + 128]

            def pws_w(gi, go):
                o = O_PWS + (gi * G + go) * 128
                return pbf[:, o:o + 128]

            ID = pbf[:, O_ID:O_ID + 128]

            # ---- kernel branch, all 16 samples batched (free dim 400) ----
            K2 = []
            with tc.tile_pool(name="psk", bufs=2, space="PSUM") as pskp:
                hks = []
                for g in range(G):
                    xk = kpool.tile([128, BPC, KH, KH], BF16, name=f"xk{g}")
                    nc.sync.dma_start(out=xk[:], in_=kern_h[128 * g:128 * (g + 1)])
                    psd = pskp.tile([128, NKF], F32, name="pskd")
                    for t in range(9):
                        u, v = t // 3, t % 3
                        nc.tensor.matmul(
                            psd[:].rearrange("p (s a b) -> p s a b", s=BPC, a=5),
                            dwk_w(g, t), xk[:, :, u:u + 5, v:v + 5],
                            start=(t == 0), stop=(t == 8))
                    hk0 = kpool.tile([128, NKF], BF16, name=f"hk0{g}")
                    nc.scalar.activation(hk0[:], psd[:], AF.Relu,
                                         bias=bias(0 + g), scale=1.0)
                    hk = kpool.tile([128, NKF], BF16, name=f"hk{g}")
                    nc.vector.tensor_scalar(hk[:], hk0[:], 6.0, None, OP.min)
                    hks.append(hk)
                for go in range(G):
                    psp = pskp.tile([128, NKF], F32, name="pskp")
                    for gi in range(G):
                        nc.tensor.matmul(psp[:], pwk_w(gi, go), hks[gi][:],
                                         start=(gi == 0), stop=(gi == 1))
                    k2 = kpool.tile([128, NKF], F32, name=f"k2{go}")
                    nc.scalar.activation(k2[:], psp[:], AF.Identity,
                                         bias=bias(2 + go), scale=1.0)
                    K2.append(k2)

            # ---- search branch + xcorr, per sample ----
            with (
                tc.tile_pool(name="sio", bufs=3) as spool,
                tc.tile_pool(name="hp", bufs=2) as hpool,
                tc.tile_pool(name="s2p", bufs=2) as s2pool,
                tc.tile_pool(name="dg", bufs=2) as dpool,
                tc.tile_pool(name="vx", bufs=2) as vxp,
                tc.tile_pool(name="ox", bufs=2) as oxp,
                tc.tile_pool(name="psd", bufs=1, space="PSUM") as psdp,
                tc.tile_pool(name="psp", bufs=1, space="PSUM") as pspp,
                tc.tile_pool(name="psx", bufs=2, space="PSUM") as psxp,
            ):
                for s in range(BPC):
                    # dw conv + relu6 per block
                    h2s = []
                    for g in range(G):
                        xs = spool.tile([128, SH, SW], BF16, name=f"xs{g}")
                        nc.sync.dma_start(
                            out=xs[:], in_=srch_h[128 * g:128 * (g + 1), s])
                        pcs = []
                        for (r0, r1) in DW_CH:
                            ps = psdp.tile([128, (r1 - r0) * HO_S], F32,
                                           name=f"dw{r0}")
                            pcs.append(ps)
                        for t in range(9):
                            u, v = t // 3, t % 3
                            for ci, (r0, r1) in enumerate(DW_CH):
                                nc.tensor.matmul(
                                    pcs[ci][:].rearrange(
                                        "p (a b) -> p a b", a=r1 - r0),
                                    dws_w(g, t),
                                    xs[:, u + r0:u + r1, v:v + HO_S],
                                    start=(t == 0), stop=(t == 8))
                        h = hpool.tile([128, HO_S * HO_S], BF16, name=f"h{g}")
                        o = 0
                        for ci, (r0, r1) in enumerate(DW_CH):
                            n = (r1 - r0) * HO_S
                            nc.scalar.activation(h[:, o:o + n], pcs[ci][:],
                                                 AF.Relu, bias=bias(4 + g),
                                                 scale=1.0)
                            o += n
                        h2 = hpool.tile([128, HO_S * HO_S], BF16, name=f"h2{g}")
                        nc.vector.tensor_scalar(h2[:], h[:], 6.0, None, OP.min)
                        h2s.append(h2)

                    # pw conv -> padded S2 tiles
                    S2s = []
                    for go in range(G):
                        s2 = s2pool.tile([128, HO_S, 30], BF16, name=f"s2{go}")
                        for (c0, c1) in PW_CH:
                            ps = pspp.tile([128, c1 - c0], F32, name=f"pw{c0}")
                            for gi in range(G):
                                nc.tensor.matmul(ps[:], pws_w(gi, go),
                                                 h2s[gi][:, c0:c1],
                                                 start=(gi == 0), stop=(gi == 1))
                            r0, r1 = c0 // HO_S, c1 // HO_S
                            nc.scalar.activation(
                                s2[:, r0:r1, 0:HO_S],
                                ps[:].rearrange("p (a b) -> p a b", a=r1 - r0),
                                AF.Identity, bias=bias(6 + go), scale=1.0)
                        S2s.append(s2)

                    # xcorr per block
                    for g in range(G):
                        idx = s * 2 + g
                        route = ROUTES[idx]
                        s2 = S2s[g]

                        def k2c(t):
                            return K2[g][:, s * 25 + t:s * 25 + t + 1]

                        if route == 'P':
                            # support work (diag builds, eviction) goes on the
                            # engine OPPOSITE the co-tile's route, so it never
                            # queues behind a 20us tap-chain/product block
                            co = ROUTES[s * 2 + (1 - g)]
                            use_v = (co == 'W') or (co == 'P' and g == 0)
                            diags = []
                            for t in range(25):
                                d = dpool.tile([128, 128], BF16, name=f"d{t}")
                                if use_v:
                                    nc.vector.tensor_scalar(
                                        d[:], ID, k2c(t), None, OP.mult)
                                else:
                                    nc.scalar.activation(
                                        d[:], ID, AF.Identity, bias=0.0,
                                        scale=k2c(t))
                                diags.append(d)
                            pcs = []
                            for (r0, r1) in X_CH:
                                pcs.append(psxp.tile(
                                    [128, (r1 - r0) * HO_X], F32,
                                    name=f"x{r0}"))
                            for t in range(25):
                                u, v = t // 5, t % 5
                                for ci, (r0, r1) in enumerate(X_CH):
                                    nc.tensor.matmul(
                                        pcs[ci][:].rearrange(
                                            "p (a b) -> p a b", a=r1 - r0),
                                        diags[t][:],
                                        s2[:, u + r0:u + r1, v:v + HO_X],
                                        start=(t == 0), stop=(t == 24))
                            oxf = oxp.tile([128, HO_X, HO_X], F32, name="oxf")
                            for ci, (r0, r1) in enumerate(X_CH):
                                pview = pcs[ci][:].rearrange(
                                    "p (a b) -> p a b", a=r1 - r0)
                                if use_v:
                                    nc.vector.tensor_copy(
                                        oxf[:, r0:r1, :], pview)
                                else:
                                    nc.scalar.activation(
                                        oxf[:, r0:r1, :], pview,
                                        AF.Identity, bias=0.0, scale=1.0)
                            nc.sync.dma_start(
                                out=out_h[128 * g:128 * (g + 1), s],
                                in_=oxf[:])
                        elif route == 'V':
                            sh = s2pool.tile([128, HO_S, 30], BF16,
                                             name=f"sh{g}")
                            nc.vector.tensor_copy(sh[:, :, 0:28],
                                                  s2[:, :, 1:29])
                            accs = []
                            for ci, (t0, t1) in enumerate(V_CHAINS):
                                a = vxp.tile([128, HO_X, HO_X], BF16,
                                             name=f"va{ci}")
                                for t in range(t0, t1):
                                    u, v = t // 5, t % 5
                                    if v % 2 == 0:
                                        win = s2[:, u:u + 25, v:v + 25]
                                    else:
                                        win = sh[:, u:u + 25, v - 1:v + 24]
                                    if t == t0:
                                        nc.vector.tensor_scalar(
                                            a[:], win, k2c(t), None, OP.mult)
                                    else:
                                        nc.vector.scalar_tensor_tensor(
                                            a[:], win, k2c(t), a[:],
                                            OP.mult, OP.add)
                                accs.append(a)
                            c01 = vxp.tile([128, HO_X, HO_X], BF16, name="c01")
                            nc.vector.tensor_tensor(
                                c01[:], accs[0][:], accs[1][:], OP.add)
                            c23 = vxp.tile([128, HO_X, HO_X], BF16, name="c23")
                            nc.vector.tensor_tensor(
                                c23[:], accs[2][:], accs[3][:], OP.add)
                            ovf = oxp.tile([128, HO_X, HO_X], F32, name="ovf")
                            nc.vector.tensor_tensor(
                                ovf[:], c01[:], c23[:], OP.add)
                            nc.sync.dma_start(
                                out=out_h[128 * g:128 * (g + 1), s],
                                in_=ovf[:])
                        else:  # 'W': ScalarE products + VectorE tree-sum
                            prods = []
                            for t in range(25):
                                u, v = t // 5, t % 5
                                pr = vxp.tile([128, HO_X * HO_X], BF16,
                                              name=f"pr{t % 8}")
                                nc.scalar.activation(
                                    pr[:].rearrange("p (a b) -> p a b", a=25),
                                    s2[:, u:u + 25, v:v + 25],
                                    AF.Identity, bias=0.0, scale=k2c(t))
                                prods.append(pr)
                            caccs = []
                            for ci, (t0, t1) in enumerate(V_CHAINS):
                                ca = vxp.tile([128, HO_X * HO_X], BF16,
                                              name=f"wc{ci}")
                                nc.vector.tensor_tensor(
                                    ca[:], prods[t0][:], prods[t0 + 1][:],
                                    OP.add)
                                for t in range(t0 + 2, t1):
                                    nc.vector.tensor_tensor(
                                        ca[:], ca[:], prods[t][:], OP.add)
                                caccs.append(ca)
                            c01 = vxp.tile([128, HO_X * HO_X], BF16,
                                           name="wc01")
                            nc.vector.tensor_tensor(
                                c01[:], caccs[0][:], caccs[1][:], OP.add)
                            c23 = vxp.tile([128, HO_X * HO_X], BF16,
                                           name="wc23")
                            nc.vector.tensor_tensor(
                                c23[:], caccs[2][:], caccs[3][:], OP.add)
                            ovf = oxp.tile([128, HO_X, HO_X], F32, name="ovf")
                            nc.vector.tensor_tensor(
                                ovf[:].rearrange("p a b -> p (a b)"),
                                c01[:], c23[:], OP.add)
                            nc.sync.dma_start(
                                out=out_h[128 * g:128 * (g + 1), s],
                                in_=ovf[:])
    _split_waits(nc)
    return nc


def _pack_params(kdw, ks1, kpw, kb2, sdw, ss1, spw, sb2):
    pbf = np.zeros((128, NBF), np.float32)
    for g in range(G):
        for t in range(9):
            d = np.diag(kdw[g * 128:(g + 1) * 128, t])
            pbf[:, O_DWK + (g * 9 + t) * 128:O_DWK + (g * 9 + t + 1) * 128] = d
            d = np.diag(sdw[g * 128:(g + 1) * 128, t])
            pbf[:, O_DWS + (g * 9 + t) * 128:O_DWS + (g * 9 + t + 1) * 128] = d
    for gi in range(G):
        for go in range(G):
            o = O_PWK + (gi * G + go) * 128
            pbf[:, o:o + 128] = kpw[gi, go]
            o = O_PWS + (gi * G + go) * 128
            pbf[:, o:o + 128] = spw[gi, go]
    pbf[:, O_ID:O_ID + 128] = np.eye(128, dtype=np.float32)

    p32 = np.zeros((128, NP32), np.float32)
    for g in range(G):
        p32[:, 0 + g] = ks1[g * 128:(g + 1) * 128]
        p32[:, 2 + g] = kb2[g * 128:(g + 1) * 128]
        p32[:, 4 + g] = ss1[g * 128:(g + 1) * 128]
        p32[:, 6 + g] = sb2[g * 128:(g + 1) * 128]
    return pbf.astype(BF_NP), p32


def kernel(kernel, search, k_dw_w, k_bn1, k_pw_w, k_pw_b, k_bn2,
           s_dw_w, s_bn1, s_pw_w, s_pw_b, s_bn2):
    global LAST_RESULTS
    kdw, ks1, kpw, kb2 = _fold_branch(np.asarray(k_dw_w), np.asarray(k_bn1),
                                      np.asarray(k_pw_w), np.asarray(k_pw_b),
                                      np.asarray(k_bn2))
    sdw, ss1, spw, sb2 = _fold_branch(np.asarray(s_dw_w), np.asarray(s_bn1),
                                      np.asarray(s_pw_w), np.asarray(s_pw_b),
                                      np.asarray(s_bn2))
    pbf, p32 = _pack_params(kdw, ks1, kpw, kb2, sdw, ss1, spw, sb2)

    kern = np.asarray(kernel, np.float32)
    srch = np.asarray(search, np.float32)
    # channel-major per-core layouts, search col-padded 31->32
    kern_cm = np.ascontiguousarray(
        kern.reshape(N_CORES, BPC, C, KH, KH).transpose(0, 2, 1, 3, 4)
    ).astype(BF_NP)
    srch_p = np.zeros((N_CORES, C, BPC, SH, SW), np.float32)
    srch_p[..., :SH] = srch.reshape(N_CORES, BPC, C, SH, SH).transpose(
        0, 2, 1, 3, 4)
    srch_cm = srch_p.astype(BF_NP)

    if "nc" not in _cache:
        _cache["nc"] = _build_nc()
    nc = _cache["nc"]

    in_maps = []
    for i in range(N_CORES):
        in_maps.append({"kern_in": kern_cm[i], "srch_in": srch_cm[i],
                        "prmbf": pbf, "prm32": p32})

    res = run_bass_kernel_spmd(nc, in_maps, list(range(N_CORES)))
    LAST_RESULTS = res
    outs = []
    for i in range(N_CORES):
        o = res.results[i]["out"]          # [C, BPC, 25, 25]
        outs.append(np.ascontiguousarray(o.transpose(1, 0, 2, 3)))
    return np.concatenate(outs, axis=0)


# revision 14
# speedup vs baseline: 1.1000x; 1.0508x over previous
"""DepthwiseXCorr (SiamRPN head) on 8 trn2 cores — PE-centric bf16 rewrite.

Data-parallel over batch: B=128 -> 16 samples/core. Per sample:
  branch(x) = BN2(pw1x1(ReLU6(BN1(dw3x3(x)))))  for kernel (7x7) and search (31x31)
  out = per-channel xcorr(search_feat 29x29, kernel_feat 5x5) -> 25x25

Engine mapping (per core):
  - dw conv (both branches): PE matmuls with HOST-PRECOMPUTED diagonal weight
    matrices (bf16), accumulating 9 taps in PSUM (fp32). Kernel branch is
    batched across all 16 samples (free dim 400).
  - BN bias + ReLU: fused into ScalarE PSUM eviction (Relu, bias=b1).
    min(.,6) on VectorE (tensor_scalar with immediate hits the 2x mode).
  - pw conv: PE bf16 matmuls (BN2 folded into weights), ScalarE eviction
    with bias into padded [29,30] bf16 feature tiles.
  - xcorr: 32 tiles (sample x channel-block) routed across engines:
      'P': PE diag-matmuls; diagonals built from an identity via
           per-partition scale (split VectorE/ScalarE); fp32 PSUM accum.
      'V': VectorE scalar_tensor_tensor taps in 4 short bf16 chains
           (7/6/6/6) combined in fp32 (keeps accumulation error low);
           odd-column windows read a 1-shifted copy to stay 4B-aligned.
      'W': ScalarE per-tap products (activation with per-partition
           scale) + VectorE tree-sum of the contiguous product tiles.
  - outputs DMA'd from SBUF fp32 tiles.

Host-side layouts are channel-major so every DMA is contiguous per
partition: kern [256,16,7,7], srch [256,16,31,33] (col-padded to an odd
row stride: 66B avoids the SBUF port conflicts a 64B stride causes for
PE moving-operand streaming), out [256,16,25,25].
"""

import numpy as np
import ml_dtypes

import concourse.bass as bass
import concourse.mybir as mybir
from concourse.tile import TileContext
from concourse.bass_utils import run_bass_kernel_spmd

F32 = mybir.dt.float32
BF16 = mybir.dt.bfloat16
AF = mybir.ActivationFunctionType
OP = mybir.AluOpType
BF_NP = ml_dtypes.bfloat16

B, C, KH, SH = 128, 256, 7, 31
N_CORES = 8
BPC = B // N_CORES          # 16 samples per core
G = 2                       # channel blocks of 128
EPS = 1e-5
HO_K, HO_S, HO_X = 5, 29, 25
SW = 33                     # padded search row width (odd stride avoids SBUF port conflicts)
NKF = BPC * HO_K * HO_K     # 400: kernel branch batched free dim

# bf16 param column offsets: [dw diag k (2g x 9t x 128) | dw diag s |
#                             pw k lhsT (4 x 128) | pw s lhsT | identity]
O_DWK, O_DWS = 0, 2304
O_PWK, O_PWS = 4608, 5120
O_ID = 5632
NBF = 5760
# fp32 param cols: b1k g0,g1 | b2k g0,g1 | b1s g0,g1 | b2s g0,g1
NP32 = 8

# dw psum row-chunks (output rows), pw chunks (flat cols), xcorr row-chunks
DW_CH = ((0, 16), (16, 29))         # 464 / 377 elems
PW_CH = ((0, 493), (493, 841))      # 17x29 / 12x29
X_CH = ((0, 20), (20, 25))          # 500 / 125 elems
V_CHAINS = ((0, 7), (7, 13), (13, 19), (19, 25))


def _routes():
    # Three xcorr routes balanced across engines (measured costs):
    #   'P': PE diag-matmuls (~11us PE) + ScalarE diag builds
    #   'V': VectorE scalar_tensor_tensor taps (~21us V)
    #   'W': ScalarE per-tap products (activation, per-partition scale) +
    #        VectorE tree-sum of contiguous product tiles
    # One PE tile in every sample so the Tensor engine never drains (HAM
    # re-throttles after idle windows); second tile alternates V/W, with two
    # extra P tiles to balance measured engine loads (PE 254 / V 352 / S 324).
    r = []
    for s in range(16):
        r.append('P')
        if s in (0, 8):
            r.append('P')
        elif s % 2 == 1:
            r.append('V')
        else:
            r.append('W')
    return r


ROUTES = _routes()

_cache: dict = {}
LAST_RESULTS = None


def _fold_branch(dw_w, bn1, pw_w, pw_b, bn2):
    """Fold eval-mode BN into conv weights/biases (host, fp32)."""
    g1, b1, m1, v1 = bn1[0], bn1[1], bn1[2], bn1[3]
    inv1 = g1 / np.sqrt(v1 + EPS)
    shift1 = b1 - m1 * inv1
    dw = (dw_w[:, 0] * inv1[:, None, None]).reshape(C, 9).astype(np.float32)

    g2, b2, m2, v2 = bn2[0], bn2[1], bn2[2], bn2[3]
    inv2 = g2 / np.sqrt(v2 + EPS)
    shift2 = b2 - m2 * inv2
    W = (pw_w[:, :, 0, 0] * inv2[:, None]).astype(np.float32)   # (co, ci)
    bias2 = (pw_b * inv2 + shift2).astype(np.float32)

    lhsT = np.zeros((G, G, 128, 128), np.float32)
    for gi in range(G):
        for go in range(G):
            lhsT[gi, go] = W[go * 128:(go + 1) * 128, gi * 128:(gi + 1) * 128].T
    return dw, shift1.astype(np.float32), lhsT, bias2


def _split_waits(nc, keep=1):
    """Container walrus accepts only one sync-wait per instruction; move
    extras onto standalone EventSemaphore instructions just before the
    owner in its engine stream."""
    import bass_rust

    n = 0
    for bb in nc.m.functions[0].blocks:
        out = []
        for ins in bb.instructions:
            si = ins.sync_info
            if si is not None and len(si.on_wait) > keep:
                waits = list(si.on_wait)
                for w in waits[:-keep]:
                    n += 1
                    ev = mybir.InstEventSemaphore(
                        name=f"antsplitw_{n}", ins=[], outs=[])
                    ev.engine = ins.engine
                    ev.sync_info = bass_rust.SyncInfo(on_wait=[w], on_update=[])
                    out.append(ev)
                ins.sync_info = bass_rust.SyncInfo(
                    on_wait=waits[-keep:], on_update=list(si.on_update))
            out.append(ins)
        bb.instructions = out
    return n


def _build_nc():
    nc = bass.Bass()

    kern_h = nc.declare_dram_parameter("kern_in", [C, BPC, KH, KH], BF16, isOutput=False)
    srch_h = nc.declare_dram_parameter("srch_in", [C, BPC, SH, SW], BF16, isOutput=False)
    pbf_h = nc.declare_dram_parameter("prmbf", [128, NBF], BF16, isOutput=False)
    p32_h = nc.declare_dram_parameter("prm32", [128, NP32], F32, isOutput=False)
    out_h = nc.declare_dram_parameter("out", [C, BPC, HO_X, HO_X], F32, isOutput=True)

    with TileContext(nc) as tc:
        with (
            tc.tile_pool(name="const", bufs=1) as cpool,
            tc.tile_pool(name="kio", bufs=1) as kpool,
        ):
            pbf = cpool.tile([128, NBF], BF16)
            nc.sync.dma_start(out=pbf[:], in_=pbf_h[:])
            p32 = cpool.tile([128, NP32], F32)
            nc.sync.dma_start(out=p32[:], in_=p32_h[:])

            def bias(i):
                return p32[:, i:i + 1]

            def dwk_w(g, t):
                o = O_DWK + (g * 9 + t) * 128
                return pbf[:, o:o + 128]

            def dws_w(g, t):
                o = O_DWS + (g * 9 + t) * 128
                return pbf[:, o:o + 128]

            def pwk_w(gi, go):
                o = O_PWK + (gi * G + go) * 128
                return pbf[:, o:o + 128]

            def pws_w(gi, go):
                o = O_PWS + (gi * G + go) * 128
                return pbf[:, o:o + 128]

            ID = pbf[:, O_ID:O_ID + 128]

            # ---- kernel branch, all 16 samples batched (free dim 400) ----
            K2 = []
            with tc.tile_pool(name="psk", bufs=2, space="PSUM") as pskp:
                hks = []
                for g in range(G):
                    xk = kpool.tile([128, BPC, KH, KH], BF16, name=f"xk{g}")
                    nc.sync.dma_start(out=xk[:], in_=kern_h[128 * g:128 * (g + 1)])
                    psd = pskp.tile([128, NKF], F32, name="pskd")
                    for t in range(9):
                        u, v = t // 3, t % 3
                        nc.tensor.matmul(
                            psd[:].rearrange("p (s a b) -> p s a b", s=BPC, a=5),
                            dwk_w(g, t), xk[:, :, u:u + 5, v:v + 5],
                            start=(t == 0), stop=(t == 8))
                    hk0 = kpool.tile([128, NKF], BF16, name=f"hk0{g}")
                    nc.scalar.activation(hk0[:], psd[:], AF.Relu,
                                         bias=bias(0 + g), scale=1.0)
                    hk = kpool.tile([128, NKF], BF16, name=f"hk{g}")
                    nc.vector.tensor_scalar(hk[:], hk0[:], 6.0, None, OP.min)
                    hks.append(hk)
                for go in range(G):
                    psp = pskp.tile([128, NKF], F32, name="pskp")
                    for gi in range(G):
                        nc.tensor.matmul(psp[:], pwk_w(gi, go), hks[gi][:],
                                         start=(gi == 0), stop=(gi == 1))
                    k2 = kpool.tile([128, NKF], F32, name=f"k2{go}")
                    nc.scalar.activation(k2[:], psp[:], AF.Identity,
                                         bias=bias(2 + go), scale=1.0)
                    K2.append(k2)

            # ---- search branch + xcorr, per sample ----
            with (
                tc.tile_pool(name="sio", bufs=3) as spool,
                tc.tile_pool(name="hp", bufs=2) as hpool,
                tc.tile_pool(name="s2p", bufs=2) as s2pool,
                tc.tile_pool(name="dg", bufs=2) as dpool,
                tc.tile_pool(name="vx", bufs=2) as vxp,
                tc.tile_pool(name="ox", bufs=2) as oxp,
                tc.tile_pool(name="psd", bufs=1, space="PSUM") as psdp,
                tc.tile_pool(name="psp", bufs=1, space="PSUM") as pspp,
                tc.tile_pool(name="psx", bufs=2, space="PSUM") as psxp,
            ):
                for s in range(BPC):
                    # dw conv + relu6 per block
                    h2s = []
                    for g in range(G):
                        xs = spool.tile([128, SH, SW], BF16, name=f"xs{g}")
                        nc.sync.dma_start(
                            out=xs[:], in_=srch_h[128 * g:128 * (g + 1), s])
                        pcs = []
                        for (r0, r1) in DW_CH:
                            ps = psdp.tile([128, (r1 - r0) * HO_S], F32,
                                           name=f"dw{r0}")
                            pcs.append(ps)
                        for t in range(9):
                            u, v = t // 3, t % 3
                            for ci, (r0, r1) in enumerate(DW_CH):
                                nc.tensor.matmul(
                                    pcs[ci][:].rearrange(
                                        "p (a b) -> p a b", a=r1 - r0),
                                    dws_w(g, t),
                                    xs[:, u + r0:u + r1, v:v + HO_S],
                                    start=(t == 0), stop=(t == 8))
                        h = hpool.tile([128, HO_S * HO_S], BF16, name=f"h{g}")
                        o = 0
                        for ci, (r0, r1) in enumerate(DW_CH):
                            n = (r1 - r0) * HO_S
                            nc.scalar.activation(h[:, o:o + n], pcs[ci][:],
                                                 AF.Relu, bias=bias(4 + g),
                                                 scale=1.0)
                            o += n
                        h2 = hpool.tile([128, HO_S * HO_S], BF16, name=f"h2{g}")
                        nc.vector.tensor_scalar(h2[:], h[:], 6.0, None, OP.min)
                        h2s.append(h2)

                    # pw conv -> padded S2 tiles
                    S2s = []
                    for go in range(G):
                        s2 = s2pool.tile([128, HO_S, 30], BF16, name=f"s2{go}")
                        for (c0, c1) in PW_CH:
                            ps = pspp.tile([128, c1 - c0], F32, name=f"pw{c0}")
                            for gi in range(G):
                                nc.tensor.matmul(ps[:], pws_w(gi, go),
                                                 h2s[gi][:, c0:c1],
                                                 start=(gi == 0), stop=(gi == 1))
                            r0, r1 = c0 // HO_S, c1 // HO_S
                            nc.scalar.activation(
                                s2[:, r0:r1, 0:HO_S],
                                ps[:].rearrange("p (a b) -> p a b", a=r1 - r0),
                                AF.Identity, bias=bias(6 + go), scale=1.0)
                        S2s.append(s2)

                    # xcorr per block
                    for g in range(G):
                        idx = s * 2 + g
                        route = ROUTES[idx]
                        s2 = S2s[g]

                        def k2c(t):
                            return K2[g][:, s * 25 + t:s * 25 + t + 1]

                        if route == 'P':
                            # support work (diag builds, eviction) goes on the
                            # engine OPPOSITE the co-tile's route, so it never
                            # queues behind a 20us tap-chain/product block
                            co = ROUTES[s * 2 + (1 - g)]
                            use_v = (co == 'W') or (co == 'P' and g == 0)
                            diags = []
                            for t in range(25):
                                d = dpool.tile([128, 128], BF16, name=f"d{t}")
                                if use_v:
                                    nc.vector.tensor_scalar(
                                        d[:], ID, k2c(t), None, OP.mult)
                                else:
                                    nc.scalar.activation(
                                        d[:], ID, AF.Identity, bias=0.0,
                                        scale=k2c(t))
                                diags.append(d)
                            pcs = []
                            for (r0, r1) in X_CH:
                                pcs.append(psxp.tile(
                                    [128, (r1 - r0) * HO_X], F32,
                                    name=f"x{r0}"))
                            for t in range(25):
                                u, v = t // 5, t % 5
                                for ci, (r0, r1) in enumerate(X_CH):
                                    nc.tensor.matmul(
                                        pcs[ci][:].rearrange(
                                            "p (a b) -> p a b", a=r1 - r0),
                                        diags[t][:],
                                        s2[:, u + r0:u + r1, v:v + HO_X],
                                        start=(t == 0), stop=(t == 24))
                            oxf = oxp.tile([128, HO_X, HO_X], F32, name="oxf")
                            for ci, (r0, r1) in enumerate(X_CH):
                                pview = pcs[ci][:].rearrange(
                                    "p (a b) -> p a b", a=r1 - r0)
                                if use_v:
                                    nc.vector.tensor_copy(
                                        oxf[:, r0:r1, :], pview)
                                else:
                                    nc.scalar.activation(
                                        oxf[:, r0:r1, :], pview,
                                        AF.Identity, bias=0.0, scale=1.0)
                            nc.sync.dma_start(
                                out=out_h[128 * g:128 * (g + 1), s],
                                in_=oxf[:])
                        elif route == 'V':
                            sh = s2pool.tile([128, HO_S, 30], BF16,
                                             name=f"sh{g}")
                            nc.vector.tensor_copy(sh[:, :, 0:28],
                                                  s2[:, :, 1:29])
                            accs = []
                            for ci, (t0, t1) in enumerate(V_CHAINS):
                                a = vxp.tile([128, HO_X, HO_X], BF16,
                                             name=f"va{ci}")
                                for t in range(t0, t1):
                                    u, v = t // 5, t % 5
                                    if v % 2 == 0:
                                        win = s2[:, u:u + 25, v:v + 25]
                                    else:
                                        win = sh[:, u:u + 25, v - 1:v + 24]
                                    if t == t0:
                                        nc.vector.tensor_scalar(
                                            a[:], win, k2c(t), None, OP.mult)
                                    else:
                                        nc.vector.scalar_tensor_tensor(
                                            a[:], win, k2c(t), a[:],
                                            OP.mult, OP.add)
                                accs.append(a)
                            c01 = vxp.tile([128, HO_X, HO_X], BF16, name="c01")
                            nc.vector.tensor_tensor(
                                c01[:], accs[0][:], accs[1][:], OP.add)
                            c23 = vxp.tile([128, HO_X, HO_X], BF16, name="c23")
                            nc.vector.tensor_tensor(
                                c23[:], accs[2][:], accs[3][:], OP.add)
                            ovf = oxp.tile([128, HO_X, HO_X], F32, name="ovf")
                            nc.vector.tensor_tensor(
                                ovf[:], c01[:], c23[:], OP.add)
                            nc.sync.dma_start(
                                out=out_h[128 * g:128 * (g + 1), s],
                                in_=ovf[:])
                        else:  # 'W': ScalarE products + VectorE tree-sum
                            prods = []
                            for t in range(25):
                                u, v = t // 5, t % 5
                                pr = vxp.tile([128, HO_X * HO_X], BF16,
                                              name=f"pr{t % 8}")
                                pv = pr[:].rearrange("p (a b) -> p a b", a=25)
                                if t % 3 == 2:
                                    # every 3rd product on VectorE: shortens
                                    # the ScalarE FIFO block that delays the
                                    # next sample's dw evictions
                                    nc.vector.tensor_scalar(
                                        pv, s2[:, u:u + 25, v:v + 25],
                                        k2c(t), None, OP.mult)
                                else:
                                    nc.scalar.activation(
                                        pv, s2[:, u:u + 25, v:v + 25],
                                        AF.Identity, bias=0.0, scale=k2c(t))
                                prods.append(pr)
                            caccs = []
                            for ci, (t0, t1) in enumerate(V_CHAINS):
                                ca = vxp.tile([128, HO_X * HO_X], BF16,
                                              name=f"wc{ci}")
                                nc.vector.tensor_tensor(
                                    ca[:], prods[t0][:], prods[t0 + 1][:],
                                    OP.add)
                                for t in range(t0 + 2, t1):
                                    nc.vector.tensor_tensor(
                                        ca[:], ca[:], prods[t][:], OP.add)
                                caccs.append(ca)
                            c01 = vxp.tile([128, HO_X * HO_X], BF16,
                                           name="wc01")
                            nc.vector.tensor_tensor(
                                c01[:], caccs[0][:], caccs[1][:], OP.add)
                            c23 = vxp.tile([128, HO_X * HO_X], BF16,
                                           name="wc23")
                            nc.vector.tensor_tensor(
                                c23[:], caccs[2][:], caccs[3][:], OP.add)
                            ovf = oxp.tile([128, HO_X, HO_X], F32, name="ovf")
                            nc.vector.tensor_tensor(
                                ovf[:].rearrange("p a b -> p (a b)"),
                                c01[:], c23[:], OP.add)
                            nc.sync.dma_start(
                                out=out_h[128 * g:128 * (g + 1), s],
                                in_=ovf[:])
    _split_waits(nc)
    return nc


def _pack_params(kdw, ks1, kpw, kb2, sdw, ss1, spw, sb2):
    pbf = np.zeros((128, NBF), np.float32)
    for g in range(G):
        for t in range(9):
            d = np.diag(kdw[g * 128:(g + 1) * 128, t])
            pbf[:, O_DWK + (g * 9 + t) * 128:O_DWK + (g * 9 + t + 1) * 128] = d
            d = np.diag(sdw[g * 128:(g + 1) * 128, t])
            pbf[:, O_DWS + (g * 9 + t) * 128:O_DWS + (g * 9 + t + 1) * 128] = d
    for gi in range(G):
        for go in range(G):
            o = O_PWK + (gi * G + go) * 128
            pbf[:, o:o + 128] = kpw[gi, go]
            o = O_PWS + (gi * G + go) * 128
            pbf[:, o:o + 128] = spw[gi, go]
    pbf[:, O_ID:O_ID + 128] = np.eye(128, dtype=np.float32)

    p32 = np.zeros((128, NP32), np.float32)
    for g in range(G):
        p32[:, 0 + g] = ks1[g * 128:(g + 1) * 128]
        p32[:, 2 + g] = kb2[g * 128:(g + 1) * 128]
        p32[:, 4 + g] = ss1[g * 128:(g + 1) * 128]
        p32[:, 6 + g] = sb2[g * 128:(g + 1) * 128]
    return pbf.astype(BF_NP), p32


def kernel(kernel, search, k_dw_w, k_bn1, k_pw_w, k_pw_b, k_bn2,
           s_dw_w, s_bn1, s_pw_w, s_pw_b, s_bn2):
    global LAST_RESULTS
    kdw, ks1, kpw, kb2 = _fold_branch(np.asarray(k_dw_w), np.asarray(k_bn1),
                                      np.asarray(k_pw_w), np.asarray(k_pw_b),
                                      np.asarray(k_bn2))
    sdw, ss1, spw, sb2 = _fold_branch(np.asarray(s_dw_w), np.asarray(s_bn1),
                                      np.asarray(s_pw_w), np.asarray(s_pw_b),
                                      np.asarray(s_bn2))
    pbf, p32 = _pack_params(kdw, ks1, kpw, kb2, sdw, ss1, spw, sb2)

    kern = np.asarray(kernel, np.float32)
    srch = np.asarray(search, np.float32)
    # channel-major per-core layouts, search col-padded 31->32
    kern_cm = np.ascontiguousarray(
        kern.reshape(N_CORES, BPC, C, KH, KH).transpose(0, 2, 1, 3, 4)
    ).astype(BF_NP)
    srch_p = np.zeros((N_CORES, C, BPC, SH, SW), np.float32)
    srch_p[..., :SH] = srch.reshape(N_CORES, BPC, C, SH, SH).transpose(
        0, 2, 1, 3, 4)
    srch_cm = srch_p.astype(BF_NP)

    if "nc" not in _cache:
        _cache["nc"] = _build_nc()
    nc = _cache["nc"]

    in_maps = []
    for i in range(N_CORES):
        in_maps.append({"kern_in": kern_cm[i], "srch_in": srch_cm[i],
                        "prmbf": pbf, "prm32": p32})

    res = run_bass_kernel_spmd(nc, in_maps, list(range(N_CORES)))
    LAST_RESULTS = res
    outs = []
    for i in range(N_CORES):
        o = res.results[i]["out"]          # [C, BPC, 25, 25]
        outs.append(np.ascontiguousarray(o.transpose(1, 0, 2, 3)))
    return np.concatenate(outs, axis=0)
